# revision 34
# baseline (speedup 1.0000x reference)
"""Masked dense layer  out = tanh(x @ (w*mask_w) + b*mask_b)  on 8 TRN2 cores.

Pure HBM-bandwidth problem (512 MiB f32 input, one 512->1 matvec + tanh), so
the kernel minimizes bytes streamed and keeps every engine off the critical
DMA path:

* Dead-column elimination: columns with w[f]*mask_w[f] == 0 contribute exactly
  zero, so only the K live columns ship to the device (host-side sharding /
  layout transform).  bf16 stream (rel-err ~5e-3 after tanh vs the 2e-2 gate);
  fp32 PSUM accumulation.
* Block-diagonal PE packing: per group of J=8 sub-blocks (JB=4096 rows), all
  J*K (contraction-col, sub-block) pairs are laid onto 128 PE partitions x
  NP=ceil(J*K/128) moving passes [128, 512].  The stationary [128, J] holds
  masked weights scattered one-hot by sub-block, so one PSUM bank accumulates
  the whole group as [J, 512] -- output spread across J partitions.  For
  K=272: NP=17 passes, 136 matmuls/core (~29 us PE), 8 ACTIVATEs (~5 us
  ScalarE), vs 192+64 in the naive chunked matvec.
* Host uploads x pre-permuted so every DMA is [128, span*512] with long
  per-partition contiguous runs; weights/mask are uploaded as raw-value
  scatters (wS/mS) and masked+cast on device.
"""

import numpy as np
import ml_dtypes

import concourse.bacc as bacc
import concourse.bass as bass
import concourse.tile as tile
from concourse import mybir
from concourse.bass_utils import run_bass_kernel_spmd

N, F = 262144, 512
C = 8                 # cores
R = N // C            # rows per core  = 32768
P = 128               # SBUF partitions / PE contraction rows
MM = 512              # matmul moving free dim == PSUM bank (f32)
J = 8                 # sub-blocks (output partitions) per group
JB = J * MM           # rows per group = 4096
G = R // JB           # groups per core = 8

BF16 = ml_dtypes.bfloat16

_cached = {}          # K -> (built Bass, prep metadata)


def _schedule(K: int):
    """(q, p) -> (j, k) assignment: pair index g=128q+p maps to j=g//K, k=g%K."""
    NP = (J * K + P - 1) // P
    qq, pp = np.meshgrid(np.arange(NP), np.arange(P), indexing="ij")
    gpair = qq * P + pp              # [NP, P]
    valid = gpair < J * K
    jmat = np.where(valid, gpair // K, 0)
    kmat = np.where(valid, gpair % K, 0)
    return NP, jmat, kmat, valid


def _pass_spans(NP: int):
    """Split NP passes into ~1 MiB DMA chunks (8-9 passes each)."""
    spans = []
    q = 0
    while q < NP:
        left = NP - q
        if left > 12:
            s = 8
        elif left > 9:
            s = (left + 1) // 2
        else:
            s = left
        spans.append((q, s))
        q += s
    return spans


def build_bass(K: int) -> bass.Bass:
    NP, _, _, _ = _schedule(K)
    FREE = NP * MM
    spans = _pass_spans(NP)

    nc = bacc.Bacc()

    xg = nc.declare_dram_parameter(
        "xg", [G * P, FREE], mybir.dt.bfloat16, isOutput=False
    )
    wS = nc.declare_dram_parameter("wS", [P, NP * J], mybir.dt.float32, isOutput=False)
    mS = nc.declare_dram_parameter("mS", [P, NP * J], mybir.dt.int32, isOutput=False)
    b = nc.declare_dram_parameter("b", [1], mybir.dt.float32, isOutput=False)
    mask_b = nc.declare_dram_parameter("mask_b", [1], mybir.dt.int32, isOutput=False)
    out = nc.declare_dram_parameter("out", [R, 1], mybir.dt.float32, isOutput=True)

    xg_r = xg[:, :].rearrange("(g p) f -> g p f", g=G)          # [G, P, FREE]
    out_r = out[:, :].rearrange("(g j n) one -> g j (n one)", g=G, j=J)  # [G, J, MM]

    def bcast(src_handle, parts):
        ap = src_handle[:]
        return bass.AP(tensor=ap.tensor, offset=ap.offset, ap=[[0, parts], [1, 1]])

    with tile.TileContext(nc) as tc:
        with (
            tc.tile_pool(name="singles", bufs=1) as singles,
            tc.tile_pool(name="xtiles", bufs=6) as xtiles,
            tc.tile_pool(name="stages", bufs=3) as stages,
            tc.tile_pool(name="psum", bufs=4, space="PSUM") as psum,
        ):
            # stationary weights: lhsT[p, 8q+j] = bf16(wS * mS) (mask mult on DVE).
            # Tiny prep DMAs lead ScalarE's HWDGE ring (queue 10) so lhsT is
            # ready early while the x stream starts immediately on Sync's ring
            # (queue 1), which it has to itself.
            w_f32 = singles.tile([P, NP * J], mybir.dt.float32)
            nc.scalar.dma_start(out=w_f32, in_=wS[:, :])
            m_i32 = singles.tile([P, NP * J], mybir.dt.int32)
            nc.scalar.dma_start(out=m_i32, in_=mS[:, :])
            m_f32 = singles.tile([P, NP * J], mybir.dt.float32)
            nc.vector.tensor_copy(m_f32, m_i32)
            wm = singles.tile([P, NP * J], mybir.dt.float32)
            nc.vector.tensor_mul(wm, w_f32, m_f32)
            lhsT = singles.tile([P, NP * J], mybir.dt.bfloat16)
            nc.vector.tensor_copy(lhsT, wm)

            # masked bias on partitions 0..J-1: bm[p,0] = b[0]*mask_b[0]
            bb = singles.tile([J, 1], mybir.dt.float32)
            nc.scalar.dma_start(out=bb, in_=bcast(b, J))
            mbi = singles.tile([J, 1], mybir.dt.int32)
            nc.scalar.dma_start(out=mbi, in_=bcast(mask_b, J))
            mb = singles.tile([J, 1], mybir.dt.float32)
            nc.vector.tensor_copy(mb, mbi)
            bm = singles.tile([J, 1], mybir.dt.float32)
            nc.vector.tensor_mul(bm, bb, mb)

            for g in range(G):
                # Last group: finer trailing chunks so the post-last-byte tail
                # (matmuls of the final chunk) is short.
                gspans = spans
                if g == G - 1 and spans[-1][1] >= 6:
                    q0l, sl = spans[-1]
                    s1 = (sl + 1) // 2
                    gspans = spans[:-1] + [(q0l, s1), (q0l + s1, sl - s1)]
                tiles = []
                for q0, s in gspans:
                    bufs = 1 if g == G - 1 and (q0, s) not in spans else None
                    if bufs:
                        t = xtiles.tile(
                            [P, s * MM], mybir.dt.bfloat16, tag=f"xtt{s}", bufs=1
                        )
                    else:
                        t = xtiles.tile([P, s * MM], mybir.dt.bfloat16, tag=f"xt{s}")
                    nc.sync.dma_start(
                        out=t, in_=xg_r[g, :, q0 * MM : (q0 + s) * MM]
                    )
                    tiles.append((q0, s, t))
                ps = psum.tile([J, MM], mybir.dt.float32, tag="ps")
                for q0, s, t in tiles:
                    for qi in range(s):
                        q = q0 + qi
                        nc.tensor.matmul(
                            ps,
                            lhsT[:, q * J : (q + 1) * J],
                            t[:, qi * MM : (qi + 1) * MM],
                            start=(q == 0),
                            stop=(q == NP - 1),
                        )
                stage = stages.tile([J, MM], mybir.dt.float32, tag="stage")
                nc.scalar.activation(
                    out=stage,
                    in_=ps,
                    func=mybir.ActivationFunctionType.Tanh,
                    bias=bm,
                    scale=1.0,
                )
                # Output rides ScalarE's ring, off the x stream's Sync ring.
                nc.scalar.dma_start(out=out_r[g, :, :], in_=stage)

    nc.finalize()
    return nc


# revision 35
# speedup vs baseline: 1.1243x; 1.1243x over previous
"""Masked dense layer  out = tanh(x @ (w*mask_w) + b*mask_b)  on 8 TRN2 cores.

Pure HBM-bandwidth problem (512 MiB f32 input, one 512->1 matvec + tanh), so
the kernel minimizes bytes streamed and keeps every engine off the critical
DMA path:

* Dead-column elimination: columns with w[f]*mask_w[f] == 0 contribute exactly
  zero, so only the K live columns ship to the device (host-side sharding /
  layout transform).  bf16 stream (rel-err ~5e-3 after tanh vs the 2e-2 gate);
  fp32 PSUM accumulation.
* Block-diagonal PE packing: per group of J=8 sub-blocks (JB=4096 rows), all
  J*K (contraction-col, sub-block) pairs are laid onto 128 PE partitions x
  NP=ceil(J*K/128) moving passes [128, 512].  The stationary [128, J] holds
  masked weights scattered one-hot by sub-block, so one PSUM bank accumulates
  the whole group as [J, 512] -- output spread across J partitions.  For
  K=272: NP=17 passes, 136 matmuls/core (~29 us PE), 8 ACTIVATEs (~5 us
  ScalarE), vs 192+64 in the naive chunked matvec.
* Host uploads x pre-permuted so every DMA is [128, span*512] with long
  per-partition contiguous runs; weights/mask are uploaded as raw-value
  scatters (wS/mS) and masked+cast on device.
"""

import numpy as np
import ml_dtypes

import concourse.bacc as bacc
import concourse.bass as bass
import concourse.tile as tile
from concourse import mybir
from concourse.bass_utils import run_bass_kernel_spmd

N, F = 262144, 512
C = 8                 # cores
R = N // C            # rows per core  = 32768
P = 128               # SBUF partitions / PE contraction rows
MM = 512              # matmul moving free dim == PSUM bank (f32)
J = 8                 # sub-blocks (output partitions) per group
JB = J * MM           # rows per group = 4096
G = R // JB           # groups per core = 8

BF16 = ml_dtypes.bfloat16

_cached = {}          # K -> (built Bass, prep metadata)


def _schedule(K: int):
    """(q, p) -> (j, k) assignment: pair index g=128q+p maps to j=g//K, k=g%K."""
    NP = (J * K + P - 1) // P
    qq, pp = np.meshgrid(np.arange(NP), np.arange(P), indexing="ij")
    gpair = qq * P + pp              # [NP, P]
    valid = gpair < J * K
    jmat = np.where(valid, gpair // K, 0)
    kmat = np.where(valid, gpair % K, 0)
    return NP, jmat, kmat, valid


def _pass_spans(NP: int):
    """Split NP passes into ~1 MiB DMA chunks (8-9 passes each)."""
    spans = []
    q = 0
    while q < NP:
        left = NP - q
        if left > 12:
            s = 8
        elif left > 9:
            s = (left + 1) // 2
        else:
            s = left
        spans.append((q, s))
        q += s
    return spans


def build_bass(K: int) -> bass.Bass:
    NP, _, _, _ = _schedule(K)
    FREE = NP * MM
    spans = _pass_spans(NP)

    nc = bacc.Bacc()

    xg = nc.declare_dram_parameter(
        "xg", [G * P, FREE], mybir.dt.bfloat16, isOutput=False
    )
    wS = nc.declare_dram_parameter("wS", [P, NP * J], mybir.dt.float32, isOutput=False)
    mS = nc.declare_dram_parameter("mS", [P, NP * J], mybir.dt.int32, isOutput=False)
    b = nc.declare_dram_parameter("b", [1], mybir.dt.float32, isOutput=False)
    mask_b = nc.declare_dram_parameter("mask_b", [1], mybir.dt.int32, isOutput=False)
    out = nc.declare_dram_parameter("out", [R, 1], mybir.dt.float32, isOutput=True)

    xg_r = xg[:, :].rearrange("(g p) f -> g p f", g=G)          # [G, P, FREE]
    out_r = out[:, :].rearrange("(g j n) one -> g j (n one)", g=G, j=J)  # [G, J, MM]

    def bcast(src_handle, parts):
        ap = src_handle[:]
        return bass.AP(tensor=ap.tensor, offset=ap.offset, ap=[[0, parts], [1, 1]])

    with tile.TileContext(nc) as tc:
        with (
            tc.tile_pool(name="singles", bufs=1) as singles,
            tc.tile_pool(name="xtiles", bufs=4) as xtiles,
            tc.tile_pool(name="stages", bufs=3) as stages,
            tc.tile_pool(name="psum", bufs=4, space="PSUM") as psum,
        ):
            # stationary weights: lhsT[p, 8q+j] = bf16(wS * mS) (mask mult on DVE).
            # Tiny prep DMAs lead ScalarE's HWDGE ring (queue 10) so lhsT is
            # ready early while the x stream starts immediately on Sync's ring
            # (queue 1), which it has to itself.
            w_f32 = singles.tile([P, NP * J], mybir.dt.float32)
            nc.scalar.dma_start(out=w_f32, in_=wS[:, :])
            m_i32 = singles.tile([P, NP * J], mybir.dt.int32)
            nc.scalar.dma_start(out=m_i32, in_=mS[:, :])
            m_f32 = singles.tile([P, NP * J], mybir.dt.float32)
            nc.vector.tensor_copy(m_f32, m_i32)
            wm = singles.tile([P, NP * J], mybir.dt.float32)
            nc.vector.tensor_mul(wm, w_f32, m_f32)
            lhsT = singles.tile([P, NP * J], mybir.dt.bfloat16)
            nc.vector.tensor_copy(lhsT, wm)

            # masked bias on partitions 0..J-1: bm[p,0] = b[0]*mask_b[0]
            bb = singles.tile([J, 1], mybir.dt.float32)
            nc.scalar.dma_start(out=bb, in_=bcast(b, J))
            mbi = singles.tile([J, 1], mybir.dt.int32)
            nc.scalar.dma_start(out=mbi, in_=bcast(mask_b, J))
            mb = singles.tile([J, 1], mybir.dt.float32)
            nc.vector.tensor_copy(mb, mbi)
            bm = singles.tile([J, 1], mybir.dt.float32)
            nc.vector.tensor_mul(bm, bb, mb)

            for g in range(G):
                # Last group: finer trailing chunks so the post-last-byte tail
                # (matmuls of the final chunk) is short.
                gspans = spans
                if g == G - 1 and spans[-1][1] >= 6:
                    q0l, sl = spans[-1]
                    s1 = (sl + 1) // 2
                    gspans = spans[:-1] + [(q0l, s1), (q0l + s1, sl - s1)]
                tiles = []
                for q0, s in gspans:
                    bufs = 1 if g == G - 1 and (q0, s) not in spans else None
                    if bufs:
                        t = xtiles.tile(
                            [P, s * MM], mybir.dt.bfloat16, tag=f"xtt{s}", bufs=1
                        )
                    else:
                        t = xtiles.tile([P, s * MM], mybir.dt.bfloat16, tag=f"xt{s}")
                    nc.sync.dma_start(
                        out=t, in_=xg_r[g, :, q0 * MM : (q0 + s) * MM]
                    )
                    tiles.append((q0, s, t))
                ps = psum.tile([J, MM], mybir.dt.float32, tag="ps")
                for q0, s, t in tiles:
                    for qi in range(s):
                        q = q0 + qi
                        nc.tensor.matmul(
                            ps,
                            lhsT[:, q * J : (q + 1) * J],
                            t[:, qi * MM : (qi + 1) * MM],
                            start=(q == 0),
                            stop=(q == NP - 1),
                        )
                stage = stages.tile([J, MM], mybir.dt.float32, tag="stage")
                nc.scalar.activation(
                    out=stage,
                    in_=ps,
                    func=mybir.ActivationFunctionType.Tanh,
                    bias=bm,
                    scale=1.0,
                )
                # Output rides ScalarE's ring, off the x stream's Sync ring.
                nc.scalar.dma_start(out=out_r[g, :, :], in_=stage)

    nc.finalize()
    return nc


# revision 36
# speedup vs baseline: 1.1254x; 1.0011x over previous
"""Masked dense layer  out = tanh(x @ (w*mask_w) + b*mask_b)  on 8 TRN2 cores.

Pure HBM-bandwidth problem (512 MiB f32 input, one 512->1 matvec + tanh), so
the kernel minimizes bytes streamed and keeps every engine off the critical
DMA path:

* Dead-column elimination: columns with w[f]*mask_w[f] == 0 contribute exactly
  zero, so only the K live columns ship to the device (host-side sharding /
  layout transform).  bf16 stream (rel-err ~5e-3 after tanh vs the 2e-2 gate);
  fp32 PSUM accumulation.
* Block-diagonal PE packing: per group of J=8 sub-blocks (JB=4096 rows), all
  J*K (contraction-col, sub-block) pairs are laid onto 128 PE partitions x
  NP=ceil(J*K/128) moving passes [128, 512].  The stationary [128, J] holds
  masked weights scattered one-hot by sub-block, so one PSUM bank accumulates
  the whole group as [J, 512] -- output spread across J partitions.  For
  K=272: NP=17 passes, 136 matmuls/core (~29 us PE), 8 ACTIVATEs (~5 us
  ScalarE), vs 192+64 in the naive chunked matvec.
* Host uploads x pre-permuted so every DMA is [128, span*512] with long
  per-partition contiguous runs; weights/mask are uploaded as raw-value
  scatters (wS/mS) and masked+cast on device.
"""

import numpy as np
import ml_dtypes

import concourse.bacc as bacc
import concourse.bass as bass
import concourse.tile as tile
from concourse import mybir
from concourse.bass_utils import run_bass_kernel_spmd

N, F = 262144, 512
C = 8                 # cores
R = N // C            # rows per core  = 32768
P = 128               # SBUF partitions / PE contraction rows
MM = 512              # matmul moving free dim == PSUM bank (f32)
J = 8                 # sub-blocks (output partitions) per group
JB = J * MM           # rows per group = 4096
G = R // JB           # groups per core = 8

BF16 = ml_dtypes.bfloat16

_cached = {}          # K -> (built Bass, prep metadata)


def _schedule(K: int):
    """(q, p) -> (j, k) assignment: pair index g=128q+p maps to j=g//K, k=g%K."""
    NP = (J * K + P - 1) // P
    qq, pp = np.meshgrid(np.arange(NP), np.arange(P), indexing="ij")
    gpair = qq * P + pp              # [NP, P]
    valid = gpair < J * K
    jmat = np.where(valid, gpair // K, 0)
    kmat = np.where(valid, gpair % K, 0)
    return NP, jmat, kmat, valid


def _pass_spans(NP: int):
    """Split NP passes into ~1 MiB DMA chunks (8-9 passes each)."""
    spans = []
    q = 0
    while q < NP:
        left = NP - q
        if left > 12:
            s = 8
        elif left > 9:
            s = (left + 1) // 2
        else:
            s = left
        spans.append((q, s))
        q += s
    return spans


def build_bass(K: int) -> bass.Bass:
    NP, _, _, _ = _schedule(K)
    FREE = NP * MM
    spans = _pass_spans(NP)

    nc = bacc.Bacc()

    xg = nc.declare_dram_parameter(
        "xg", [G * P, FREE], mybir.dt.bfloat16, isOutput=False
    )
    wS = nc.declare_dram_parameter("wS", [P, NP * J], mybir.dt.float32, isOutput=False)
    mS = nc.declare_dram_parameter("mS", [P, NP * J], mybir.dt.int32, isOutput=False)
    b = nc.declare_dram_parameter("b", [1], mybir.dt.float32, isOutput=False)
    mask_b = nc.declare_dram_parameter("mask_b", [1], mybir.dt.int32, isOutput=False)
    out = nc.declare_dram_parameter("out", [R, 1], mybir.dt.float32, isOutput=True)

    xg_r = xg[:, :].rearrange("(g p) f -> g p f", g=G)          # [G, P, FREE]
    out_r = out[:, :].rearrange("(g j n) one -> g j (n one)", g=G, j=J)  # [G, J, MM]

    def bcast(src_handle, parts):
        ap = src_handle[:]
        return bass.AP(tensor=ap.tensor, offset=ap.offset, ap=[[0, parts], [1, 1]])

    with tile.TileContext(nc) as tc:
        with (
            tc.tile_pool(name="singles", bufs=1) as singles,
            tc.tile_pool(name="xtiles", bufs=5) as xtiles,
            tc.tile_pool(name="stages", bufs=3) as stages,
            tc.tile_pool(name="psum", bufs=4, space="PSUM") as psum,
        ):
            # stationary weights: lhsT[p, 8q+j] = bf16(wS * mS) (mask mult on DVE).
            # Tiny prep DMAs lead ScalarE's HWDGE ring (queue 10) so lhsT is
            # ready early while the x stream starts immediately on Sync's ring
            # (queue 1), which it has to itself.
            w_f32 = singles.tile([P, NP * J], mybir.dt.float32)
            nc.scalar.dma_start(out=w_f32, in_=wS[:, :])
            m_i32 = singles.tile([P, NP * J], mybir.dt.int32)
            nc.scalar.dma_start(out=m_i32, in_=mS[:, :])
            m_f32 = singles.tile([P, NP * J], mybir.dt.float32)
            nc.vector.tensor_copy(m_f32, m_i32)
            wm = singles.tile([P, NP * J], mybir.dt.float32)
            nc.vector.tensor_mul(wm, w_f32, m_f32)
            lhsT = singles.tile([P, NP * J], mybir.dt.bfloat16)
            nc.vector.tensor_copy(lhsT, wm)

            # masked bias on partitions 0..J-1: bm[p,0] = b[0]*mask_b[0]
            bb = singles.tile([J, 1], mybir.dt.float32)
            nc.scalar.dma_start(out=bb, in_=bcast(b, J))
            mbi = singles.tile([J, 1], mybir.dt.int32)
            nc.scalar.dma_start(out=mbi, in_=bcast(mask_b, J))
            mb = singles.tile([J, 1], mybir.dt.float32)
            nc.vector.tensor_copy(mb, mbi)
            bm = singles.tile([J, 1], mybir.dt.float32)
            nc.vector.tensor_mul(bm, bb, mb)

            for g in range(G):
                # Last group: finer trailing chunks so the post-last-byte tail
                # (matmuls of the final chunk) is short.
                gspans = spans
                if g == G - 1 and spans[-1][1] >= 6:
                    q0l, sl = spans[-1]
                    s1 = (sl + 1) // 2
                    gspans = spans[:-1] + [(q0l, s1), (q0l + s1, sl - s1)]
                tiles = []
                for q0, s in gspans:
                    bufs = 1 if g == G - 1 and (q0, s) not in spans else None
                    if bufs:
                        t = xtiles.tile(
                            [P, s * MM], mybir.dt.bfloat16, tag=f"xtt{s}", bufs=1
                        )
                    else:
                        t = xtiles.tile([P, s * MM], mybir.dt.bfloat16, tag=f"xt{s}")
                    nc.sync.dma_start(
                        out=t, in_=xg_r[g, :, q0 * MM : (q0 + s) * MM]
                    )
                    tiles.append((q0, s, t))
                ps = psum.tile([J, MM], mybir.dt.float32, tag="ps")
                for q0, s, t in tiles:
                    for qi in range(s):
                        q = q0 + qi
                        nc.tensor.matmul(
                            ps,
                            lhsT[:, q * J : (q + 1) * J],
                            t[:, qi * MM : (qi + 1) * MM],
                            start=(q == 0),
                            stop=(q == NP - 1),
                        )
                stage = stages.tile([J, MM], mybir.dt.float32, tag="stage")
                nc.scalar.activation(
                    out=stage,
                    in_=ps,
                    func=mybir.ActivationFunctionType.Tanh,
                    bias=bm,
                    scale=1.0,
                )
                # Output rides ScalarE's ring, off the x stream's Sync ring.
                nc.scalar.dma_start(out=out_r[g, :, :], in_=stage)

    nc.finalize()
    return nc
